# revision 35
# baseline (speedup 1.0000x reference)
"""Trainium2 Bass kernel for block-local (sparse window) attention.

Problem: B=4, S=4096, DIM=768, H=12 heads x DH=64, local window W=256.
    out = (softmax_blocklocal(mask(Q K^T / sqrt(DH))) V) @ Wff + bff

Sharding: 8 cores, core c = (batch c//2, sequence half c%2) -> 2048 tokens
per core = 8 complete 256-token blocks. Projections are per-token, attention
is block-local, FF is per-token => embarrassingly parallel, no collectives.

Per-core kernel (all feature-major to avoid transposes; bf16 matmuls):
  X^T [768,2048] (host-pretransposed bf16)
  Q^T/K^T = lhsT=Wq/Wk [dim,hd] (natural layout), rhs=X^T -> [hd,t]; bias via
    DVE per-partition tensor_scalar add on the PSUM->SBUF copy.
  V = token-major [t,hd]: lhsT=X^T chunk, rhs=Wv; key-padding mask folded in
    via per-partition multiply on the copy (V rows of masked keys zeroed).
  Attention per block is emitted in PE-tiling-mode-coherent phases (mode
  switches drain the PE array, so phases are not interleaved):
    A (64x128 row tiling, tiles T0/T8 concurrent): all 24 score matmuls
      scores^T[k,q], K=64 per head; head parities alternate so pairs pack.
    exp: one ACT op per (head, kc-pairs merged) [128,2,256], scale=1/8.
    B (128x64 col tiling, tiles T0/T1 concurrent): per head pair one
      av tile (attn unnormalized, parities in partition halves) and one
      dp tile = lhsT=mask-replicated [k,64] @ E^T -> denominator REPLICATED
      across the 64 partitions of its head's half (PE does the broadcast).
    One DVE reciprocal + one DVE multiply per pair normalizes both heads.
  out^T[o,t] = lhsT=Wff[hd,o] (natural), rhs=attn^T; bias=bff+bv@Wff (host-
  folded, exact because softmax rows sum to 1) on the ACT copy.
  Emission is software-pipelined in 4 token-quarter units:
    proj(u) -> attn(block 2u, 2u+1) -> FF(u)
  so ACT exp work overlaps PE projection/FF matmuls of neighboring units.
  Host transposes out^T back.
"""

import numpy as np
import ml_dtypes

import concourse.bass as bass
import concourse.mybir as mybir
from concourse import bacc
from concourse.tile import TileContext
from concourse.bass_utils import run_bass_kernel_spmd

B, S, DIM = 4, 4096, 768
H, DH = 12, 64
W = 256
NCORES = 8
T = (B * S) // NCORES       # 2048 tokens per core
NB = T // W                 # 8 blocks per core
NKC = T // 128              # 16 token chunks of 128 per core
DC = DIM // 128             # 6 dim chunks
HC = (H * DH) // 128        # 6 hd chunks
BF16 = mybir.dt.bfloat16
F32 = mybir.dt.float32

_nc_cache = {}


def _build_nc():
    nc = bacc.Bacc()

    xt_d = nc.declare_dram_parameter("xt", [DIM, T], BF16, isOutput=False)
    wq_d = nc.declare_dram_parameter("wq", [DIM, DIM], BF16, isOutput=False)
    wk_d = nc.declare_dram_parameter("wk", [DIM, DIM], BF16, isOutput=False)
    wv_d = nc.declare_dram_parameter("wv", [DIM, DIM], BF16, isOutput=False)
    wff_d = nc.declare_dram_parameter("wff", [DIM, DIM], BF16, isOutput=False)
    bq_d = nc.declare_dram_parameter("bq", [128, HC], F32, isOutput=False)
    bk_d = nc.declare_dram_parameter("bk", [128, HC], F32, isOutput=False)
    bffe_d = nc.declare_dram_parameter("bffe", [128, DC], F32, isOutput=False)
    # mask as 0/1: per-partition scalar [128, NKC] and 64-wide replicated bf16
    mv_d = nc.declare_dram_parameter("mv", [128, NKC], F32, isOutput=False)
    mbc_d = nc.declare_dram_parameter("mbc", [128, NKC * 64], BF16, isOutput=False)
    out_d = nc.declare_dram_parameter("out", [DIM, T], F32, isOutput=True)

    Exp = mybir.ActivationFunctionType.Exp
    Ident = mybir.ActivationFunctionType.Identity
    ADD = mybir.AluOpType.add
    MULT = mybir.AluOpType.mult

    with TileContext(nc) as tc:
        with (
            tc.tile_pool(name="const", bufs=1) as cpool,
            tc.tile_pool(name="mm", bufs=2, space="PSUM") as mm_pool,
            tc.tile_pool(name="sps", bufs=4, space="PSUM") as s_pool,
            tc.tile_pool(name="adp", bufs=2, space="PSUM") as ad_pool,
            tc.tile_pool(name="et", bufs=16) as et_pool,
            tc.tile_pool(name="nrm", bufs=6) as nrm_pool,
            tc.tile_pool(name="ob", bufs=6) as ob_pool,
        ):
            # ---- persistent SBUF tensors ----
            xt_sb = cpool.tile([128, DC, T], BF16, name="xt_sb")
            wq_sb = cpool.tile([128, DC, DIM], BF16, name="wq_sb")
            wk_sb = cpool.tile([128, DC, DIM], BF16, name="wk_sb")
            wv_sb = cpool.tile([128, DC, DIM], BF16, name="wv_sb")
            wff_sb = cpool.tile([128, HC, DIM], BF16, name="wff_sb")
            qt_sb = cpool.tile([128, HC, T], BF16, name="qt_sb")
            kt_sb = cpool.tile([128, HC, T], BF16, name="kt_sb")
            v_sb = cpool.tile([128, NKC, DIM], BF16, name="v_sb")
            at_sb = cpool.tile([128, HC, T], BF16, name="at_sb")
            bq_sb = cpool.tile([128, HC], F32, name="bq_sb")
            bk_sb = cpool.tile([128, HC], F32, name="bk_sb")
            bffe_sb = cpool.tile([128, DC], F32, name="bffe_sb")
            mv_sb = cpool.tile([128, NKC], F32, name="mv_sb")
            mbc_sb = cpool.tile([128, NKC, 64], BF16, name="mbc_sb")

            # ---- load inputs: X^T first (first matmuls need all of it) ----
            xt_v = xt_d.ap().rearrange("(c p) t -> p c t", p=128)
            wq_v = wq_d.ap().rearrange("(c p) o -> p c o", p=128)
            wk_v = wk_d.ap().rearrange("(c p) o -> p c o", p=128)
            wv_v = wv_d.ap().rearrange("(c p) o -> p c o", p=128)
            wff_v = wff_d.ap().rearrange("(c p) o -> p c o", p=128)
            # small tensors first (biases/masks gate the DVE copies and the
            # whole attention chain), then first token-quarter of X^T +
            # Wq/Wk so the first projection groups unlock early
            nc.sync.dma_start(out=bq_sb[:], in_=bq_d.ap())
            nc.sync.dma_start(out=bk_sb[:], in_=bk_d.ap())
            nc.sync.dma_start(out=mv_sb[:], in_=mv_d.ap())
            for dc in range(DC):
                nc.sync.dma_start(out=xt_sb[:, dc, 0:512], in_=xt_v[:, dc, 0:512])
            for dc in range(DC):
                nc.sync.dma_start(out=wq_sb[:, dc], in_=wq_v[:, dc])
            for dc in range(DC):
                nc.sync.dma_start(out=wk_sb[:, dc], in_=wk_v[:, dc])
            # needed only from the attention/FF stages (~30us+): after QK
            nc.sync.dma_start(
                out=mbc_sb[:], in_=mbc_d.ap().rearrange("p (c o) -> p c o", o=64)
            )
            nc.sync.dma_start(out=bffe_sb[:], in_=bffe_d.ap())
            for dc in range(DC):
                nc.sync.dma_start(out=wv_sb[:, dc], in_=wv_v[:, dc])
            for tt in range(1, 4):
                for dc in range(DC):
                    nc.sync.dma_start(
                        out=xt_sb[:, dc, tt * 512:(tt + 1) * 512],
                        in_=xt_v[:, dc, tt * 512:(tt + 1) * 512],
                    )
            for dc in range(DC):
                nc.sync.dma_start(out=wff_sb[:, dc], in_=wff_v[:, dc])

            def proj_qk(w_sb, b_sb, o_sb, tt):
                # one token-quarter of a Q^T/K^T projection: out [hd, 512]
                for hc in range(HC):
                    ps = mm_pool.tile([128, 512], F32, tag="mm", name="ps")
                    for dc in range(DC):
                        nc.tensor.matmul(
                            ps[:],
                            w_sb[:, dc, hc * 128:(hc + 1) * 128],
                            xt_sb[:, dc, tt * 512:(tt + 1) * 512],
                            start=(dc == 0),
                            stop=(dc == DC - 1),
                        )
                    nc.vector.tensor_scalar(
                        out=o_sb[:, hc, tt * 512:(tt + 1) * 512],
                        in0=ps[:],
                        scalar1=b_sb[:, hc:hc + 1],
                        scalar2=None,
                        op0=ADD,
                    )

            def proj_v(kc):
                # V token-chunk [128 tokens, 768], mask folded in.
                # dc outer / half inner so consecutive matmuls share lhsT.
                ps = [
                    mm_pool.tile([128, 384], F32, tag="mm", name="ps"),
                    mm_pool.tile([128, 384], F32, tag="mm", name="ps"),
                ]
                for dc in range(DC):
                    for half in range(2):
                        nc.tensor.matmul(
                            ps[half][:],
                            xt_sb[:, dc, kc * 128:(kc + 1) * 128],
                            wv_sb[:, dc, half * 384:(half + 1) * 384],
                            start=(dc == 0),
                            stop=(dc == DC - 1),
                        )
                for half in range(2):
                    nc.vector.tensor_scalar(
                        out=v_sb[:, kc, half * 384:(half + 1) * 384],
                        in0=ps[half][:],
                        scalar1=mv_sb[:, kc:kc + 1],
                        scalar2=None,
                        op0=MULT,
                    )

            def attn_block(blk):
                q0 = blk * 256

                # --- phase A: all scores (row-tiled 64x128, T0/T8 pack) ---
                # sps[hp][par] is a [128, 2, 256] tile (kc merged, same row
                # tile -> same bank is safe; parities get separate banks).
                sps = [[None, None] for _ in range(H // 2)]
                for hp in range(H // 2):
                    for kc in range(2):
                        k0 = q0 + kc * 128
                        for par in range(2):  # alternate T0/T8 for packing
                            hr = par * 64
                            if sps[hp][par] is None:
                                sps[hp][par] = s_pool.tile(
                                    [128, 2, 256], F32, tag="s", name="sp"
                                )
                            nc.tensor.matmul(
                                sps[hp][par][:, kc],
                                kt_sb[hr:hr + 64, hp, k0:k0 + 128],
                                qt_sb[hr:hr + 64, hp, q0:q0 + 256],
                                start=True, stop=True,
                            )
                # --- exp (ACT), one op per (head) over both kc ---
                ets = [[None, None] for _ in range(H // 2)]
                for hp in range(H // 2):
                    for par in range(2):
                        et = et_pool.tile([128, 2, 256], BF16, tag="et", name="et")
                        nc.scalar.activation(
                            et[:], sps[hp][par][:], Exp, bias=0.0, scale=0.125
                        )
                        ets[hp][par] = et
                # --- phase B: av (cols 0:256) + denominator (cols 256:512)
                # in one bank, col-tiled 128x64 T0/T1 ---
                for hp in range(H // 2):
                    ad = ad_pool.tile([128, 512], F32, tag="ad", name="ad")
                    for par in range(2):
                        hr = par * 64
                        h = 2 * hp + par
                        for kc in range(2):
                            tkc = blk * 2 + kc
                            nc.tensor.matmul(
                                ad[hr:hr + 64, 0:256],
                                v_sb[:, tkc, h * 64:(h + 1) * 64],
                                ets[hp][par][:, kc],
                                start=(kc == 0), stop=(kc == 1),
                            )
                    for par in range(2):
                        hr = par * 64
                        for kc in range(2):
                            tkc = blk * 2 + kc
                            nc.tensor.matmul(
                                ad[hr:hr + 64, 256:512],
                                mbc_sb[:, tkc],
                                ets[hp][par][:, kc],
                                start=(kc == 0), stop=(kc == 1),
                            )
                    rc = nrm_pool.tile([128, 256], F32, tag="rc", name="rc")
                    nc.vector.reciprocal_approx_fast(rc[:], ad[:, 256:512])
                    nc.vector.tensor_mul(
                        at_sb[:, hp, q0:q0 + 256], ad[:, 0:256], rc[:]
                    )

            def ff(tt):
                for oc in range(DC):
                    ps = mm_pool.tile([128, 512], F32, tag="mm", name="ps")
                    for hc in range(HC):
                        nc.tensor.matmul(
                            ps[:],
                            wff_sb[:, hc, oc * 128:(oc + 1) * 128],
                            at_sb[:, hc, tt * 512:(tt + 1) * 512],
                            start=(hc == 0),
                            stop=(hc == HC - 1),
                        )
                    ob = ob_pool.tile([128, 512], F32, tag="ob", name="ob")
                    nc.scalar.activation(
                        ob[:], ps[:], Ident, bias=bffe_sb[:, oc:oc + 1], scale=1.0
                    )
                    if tt == 3:
                        for fh in range(2):
                            f0 = tt * 512 + fh * 256
                            nc.sync.dma_start(
                                out=out_d.ap()[oc * 128:(oc + 1) * 128,
                                               f0:f0 + 256],
                                in_=ob[:, fh * 256:(fh + 1) * 256],
                            )
                    else:
                        nc.sync.dma_start(
                            out=out_d.ap()[oc * 128:(oc + 1) * 128,
                                           tt * 512:(tt + 1) * 512],
                            in_=ob[:],
                        )

            # ---- software-pipelined emission over 4 token-quarters ----
            for u in range(4):
                proj_qk(wq_sb, bq_sb, qt_sb, u)
                proj_qk(wk_sb, bk_sb, kt_sb, u)
                for kc in range(4 * u, 4 * u + 4):
                    proj_v(kc)
                attn_block(2 * u)
                attn_block(2 * u + 1)
                ff(u)

    nc.finalize()
    return nc


def _get_nc():
    if "nc" not in _nc_cache:
        _nc_cache["nc"] = _build_nc()
    return _nc_cache["nc"]


def _prep_in_maps(X, mask, Wq, bq, Wk, bk, Wv, bv, Wff, bff):
    bf = ml_dtypes.bfloat16
    wq_b = np.ascontiguousarray(Wq.astype(bf))
    wk_b = np.ascontiguousarray(Wk.astype(bf))
    wv_b = np.ascontiguousarray(Wv.astype(bf))
    wff_b = np.ascontiguousarray(Wff.astype(bf))
    # per-partition bias layouts: [128, nchunks] with col = chunk
    bq_t = np.ascontiguousarray(bq.astype(np.float32).reshape(HC, 128).T)
    bk_t = np.ascontiguousarray(bk.astype(np.float32).reshape(HC, 128).T)
    bffe = (bff.astype(np.float64)
            + bv.astype(np.float64) @ Wff.astype(np.float64)).astype(np.float32)
    bffe_t = np.ascontiguousarray(bffe.reshape(DC, 128).T)

    in_maps = []
    for c in range(NCORES):
        b, s0 = divmod(c, 2)
        s0 *= T
        xt = np.ascontiguousarray(X[b, s0:s0 + T, :].T.astype(bf))
        mvalid = (mask[b, s0:s0 + T] > 0).astype(np.float32)  # [T] 0/1
        mv_t = np.ascontiguousarray(mvalid.reshape(NKC, 128).T)  # [128, NKC]
        mbc = np.ascontiguousarray(
            np.broadcast_to(mv_t[:, :, None], (128, NKC, 64))
            .reshape(128, NKC * 64).astype(bf))
        in_maps.append({
            "xt": xt, "wq": wq_b, "wk": wk_b, "wv": wv_b, "wff": wff_b,
            "bq": bq_t, "bk": bk_t, "bffe": bffe_t,
            "mv": mv_t, "mbc": mbc,
        })
    return in_maps


def _assemble(results):
    out = np.empty((B, S, DIM), np.float32)
    for c in range(NCORES):
        b, s0 = divmod(c, 2)
        s0 *= T
        out[b, s0:s0 + T, :] = results[c]["out"].T
    return out


def run(trace=False, **inputs):
    nc = _get_nc()
    in_maps = _prep_in_maps(**inputs)
    res = run_bass_kernel_spmd(
        nc, in_maps, core_ids=list(range(NCORES)), trace=trace
    )
    return _assemble(res.results), res


def kernel(**inputs) -> np.ndarray:
    out, _ = run(trace=False, **inputs)
    return out


# revision 36
# speedup vs baseline: 1.0036x; 1.0036x over previous
"""Trainium2 Bass kernel for block-local (sparse window) attention.

Problem: B=4, S=4096, DIM=768, H=12 heads x DH=64, local window W=256.
    out = (softmax_blocklocal(mask(Q K^T / sqrt(DH))) V) @ Wff + bff

Sharding: 8 cores, core c = (batch c//2, sequence half c%2) -> 2048 tokens
per core = 8 complete 256-token blocks. Projections are per-token, attention
is block-local, FF is per-token => embarrassingly parallel, no collectives.

Per-core kernel (all feature-major to avoid transposes; bf16 matmuls):
  X^T [768,2048] (host-pretransposed bf16)
  Q^T/K^T = lhsT=Wq/Wk [dim,hd] (natural layout), rhs=X^T -> [hd,t]; bias via
    DVE per-partition tensor_scalar add on the PSUM->SBUF copy.
  V = token-major [t,hd]: lhsT=X^T chunk, rhs=Wv; key-padding mask folded in
    via per-partition multiply on the copy (V rows of masked keys zeroed).
  Attention per block is emitted in PE-tiling-mode-coherent phases (mode
  switches drain the PE array, so phases are not interleaved):
    A (64x128 row tiling, tiles T0/T8 concurrent): all 24 score matmuls
      scores^T[k,q], K=64 per head; head parities alternate so pairs pack.
    exp: one ACT op per (head, kc-pairs merged) [128,2,256], scale=1/8.
    B (128x64 col tiling, tiles T0/T1 concurrent): per head pair one
      av tile (attn unnormalized, parities in partition halves) and one
      dp tile = lhsT=mask-replicated [k,64] @ E^T -> denominator REPLICATED
      across the 64 partitions of its head's half (PE does the broadcast).
    One DVE reciprocal + one DVE multiply per pair normalizes both heads.
  out^T[o,t] = lhsT=Wff[hd,o] (natural), rhs=attn^T; bias=bff+bv@Wff (host-
  folded, exact because softmax rows sum to 1) on the ACT copy.
  Emission is software-pipelined in 4 token-quarter units:
    proj(u) -> attn(block 2u, 2u+1) -> FF(u)
  so ACT exp work overlaps PE projection/FF matmuls of neighboring units.
  Host transposes out^T back.
"""

import numpy as np
import ml_dtypes

import concourse.bass as bass
import concourse.mybir as mybir
from concourse import bacc
from concourse.tile import TileContext
from concourse.bass_utils import run_bass_kernel_spmd

B, S, DIM = 4, 4096, 768
H, DH = 12, 64
W = 256
NCORES = 8
T = (B * S) // NCORES       # 2048 tokens per core
NB = T // W                 # 8 blocks per core
NKC = T // 128              # 16 token chunks of 128 per core
DC = DIM // 128             # 6 dim chunks
HC = (H * DH) // 128        # 6 hd chunks
BF16 = mybir.dt.bfloat16
F32 = mybir.dt.float32

_nc_cache = {}


def _build_nc():
    nc = bacc.Bacc()

    xt_d = nc.declare_dram_parameter("xt", [DIM, T], BF16, isOutput=False)
    wq_d = nc.declare_dram_parameter("wq", [DIM, DIM], BF16, isOutput=False)
    wk_d = nc.declare_dram_parameter("wk", [DIM, DIM], BF16, isOutput=False)
    wv_d = nc.declare_dram_parameter("wv", [DIM, DIM], BF16, isOutput=False)
    wff_d = nc.declare_dram_parameter("wff", [DIM, DIM], BF16, isOutput=False)
    bq_d = nc.declare_dram_parameter("bq", [128, HC], F32, isOutput=False)
    bk_d = nc.declare_dram_parameter("bk", [128, HC], F32, isOutput=False)
    bffe_d = nc.declare_dram_parameter("bffe", [128, DC], F32, isOutput=False)
    # mask as 0/1: per-partition scalar [128, NKC] and 64-wide replicated bf16
    mv_d = nc.declare_dram_parameter("mv", [128, NKC], F32, isOutput=False)
    mbc_d = nc.declare_dram_parameter("mbc", [128, NKC * 64], BF16, isOutput=False)
    out_d = nc.declare_dram_parameter("out", [DIM, T], F32, isOutput=True)

    Exp = mybir.ActivationFunctionType.Exp
    Ident = mybir.ActivationFunctionType.Identity
    ADD = mybir.AluOpType.add
    MULT = mybir.AluOpType.mult

    with TileContext(nc) as tc:
        with (
            tc.tile_pool(name="const", bufs=1) as cpool,
            tc.tile_pool(name="mm", bufs=2, space="PSUM") as mm_pool,
            tc.tile_pool(name="sps", bufs=4, space="PSUM") as s_pool,
            tc.tile_pool(name="adp", bufs=2, space="PSUM") as ad_pool,
            tc.tile_pool(name="et", bufs=14) as et_pool,
            tc.tile_pool(name="nrm", bufs=4) as nrm_pool,
            tc.tile_pool(name="ob", bufs=3) as ob_pool,
        ):
            # ---- persistent SBUF tensors ----
            xt_sb = cpool.tile([128, DC, T], BF16, name="xt_sb")
            wq_sb = cpool.tile([128, DC, DIM], BF16, name="wq_sb")
            wk_sb = cpool.tile([128, DC, DIM], BF16, name="wk_sb")
            wv_sb = cpool.tile([128, DC, DIM], BF16, name="wv_sb")
            wff_sb = cpool.tile([128, HC, DIM], BF16, name="wff_sb")
            qt_sb = cpool.tile([128, HC, T], BF16, name="qt_sb")
            kt_sb = cpool.tile([128, HC, T], BF16, name="kt_sb")
            v_sb = cpool.tile([128, NKC, DIM], BF16, name="v_sb")
            at_sb = cpool.tile([128, HC, T], BF16, name="at_sb")
            bq_sb = cpool.tile([128, HC], F32, name="bq_sb")
            bk_sb = cpool.tile([128, HC], F32, name="bk_sb")
            bffe_sb = cpool.tile([128, DC], F32, name="bffe_sb")
            mv_sb = cpool.tile([128, NKC], F32, name="mv_sb")
            mbc_sb = cpool.tile([128, NKC, 64], BF16, name="mbc_sb")

            # ---- load inputs: X^T first (first matmuls need all of it) ----
            xt_v = xt_d.ap().rearrange("(c p) t -> p c t", p=128)
            wq_v = wq_d.ap().rearrange("(c p) o -> p c o", p=128)
            wk_v = wk_d.ap().rearrange("(c p) o -> p c o", p=128)
            wv_v = wv_d.ap().rearrange("(c p) o -> p c o", p=128)
            wff_v = wff_d.ap().rearrange("(c p) o -> p c o", p=128)
            # small tensors first (biases/masks gate the DVE copies and the
            # whole attention chain), then first token-quarter of X^T +
            # Wq/Wk so the first projection groups unlock early
            nc.sync.dma_start(out=bq_sb[:], in_=bq_d.ap())
            nc.sync.dma_start(out=bk_sb[:], in_=bk_d.ap())
            nc.sync.dma_start(out=mv_sb[:], in_=mv_d.ap())
            for dc in range(DC):
                nc.sync.dma_start(out=xt_sb[:, dc, 0:512], in_=xt_v[:, dc, 0:512])
            for dc in range(DC):
                nc.sync.dma_start(out=wq_sb[:, dc], in_=wq_v[:, dc])
            for dc in range(DC):
                nc.sync.dma_start(out=wk_sb[:, dc], in_=wk_v[:, dc])
            # needed only from the attention/FF stages (~30us+): after QK
            nc.sync.dma_start(
                out=mbc_sb[:], in_=mbc_d.ap().rearrange("p (c o) -> p c o", o=64)
            )
            nc.sync.dma_start(out=bffe_sb[:], in_=bffe_d.ap())
            for dc in range(DC):
                nc.sync.dma_start(out=wv_sb[:, dc], in_=wv_v[:, dc])
            for tt in range(1, 4):
                for dc in range(DC):
                    nc.sync.dma_start(
                        out=xt_sb[:, dc, tt * 512:(tt + 1) * 512],
                        in_=xt_v[:, dc, tt * 512:(tt + 1) * 512],
                    )
            for dc in range(DC):
                nc.sync.dma_start(out=wff_sb[:, dc], in_=wff_v[:, dc])

            def proj_qk(w_sb, b_sb, o_sb, tt):
                # one token-quarter of a Q^T/K^T projection: out [hd, 512]
                for hc in range(HC):
                    ps = mm_pool.tile([128, 512], F32, tag="mm", name="ps")
                    for dc in range(DC):
                        nc.tensor.matmul(
                            ps[:],
                            w_sb[:, dc, hc * 128:(hc + 1) * 128],
                            xt_sb[:, dc, tt * 512:(tt + 1) * 512],
                            start=(dc == 0),
                            stop=(dc == DC - 1),
                        )
                    nc.vector.tensor_scalar(
                        out=o_sb[:, hc, tt * 512:(tt + 1) * 512],
                        in0=ps[:],
                        scalar1=b_sb[:, hc:hc + 1],
                        scalar2=None,
                        op0=ADD,
                    )

            def proj_v(kc):
                # V token-chunk [128 tokens, 768], mask folded in.
                # dc outer / half inner so consecutive matmuls share lhsT.
                ps = [
                    mm_pool.tile([128, 384], F32, tag="mm", name="ps"),
                    mm_pool.tile([128, 384], F32, tag="mm", name="ps"),
                ]
                for dc in range(DC):
                    for half in range(2):
                        nc.tensor.matmul(
                            ps[half][:],
                            xt_sb[:, dc, kc * 128:(kc + 1) * 128],
                            wv_sb[:, dc, half * 384:(half + 1) * 384],
                            start=(dc == 0),
                            stop=(dc == DC - 1),
                        )
                for half in range(2):
                    nc.vector.tensor_scalar(
                        out=v_sb[:, kc, half * 384:(half + 1) * 384],
                        in0=ps[half][:],
                        scalar1=mv_sb[:, kc:kc + 1],
                        scalar2=None,
                        op0=MULT,
                    )

            def attn_block(blk):
                q0 = blk * 256

                # --- phase A: all scores (row-tiled 64x128, T0/T8 pack) ---
                # sps[hp][par] is a [128, 2, 256] tile (kc merged, same row
                # tile -> same bank is safe; parities get separate banks).
                sps = [[None, None] for _ in range(H // 2)]
                for hp in range(H // 2):
                    for kc in range(2):
                        k0 = q0 + kc * 128
                        for par in range(2):  # alternate T0/T8 for packing
                            hr = par * 64
                            if sps[hp][par] is None:
                                sps[hp][par] = s_pool.tile(
                                    [128, 2, 256], F32, tag="s", name="sp"
                                )
                            nc.tensor.matmul(
                                sps[hp][par][:, kc],
                                kt_sb[hr:hr + 64, hp, k0:k0 + 128],
                                qt_sb[hr:hr + 64, hp, q0:q0 + 256],
                                start=True, stop=True,
                            )
                # --- exp (ACT), one op per (head) over both kc ---
                ets = [[None, None] for _ in range(H // 2)]
                for hp in range(H // 2):
                    for par in range(2):
                        et = et_pool.tile([128, 2, 256], BF16, tag="et", name="et")
                        nc.scalar.activation(
                            et[:], sps[hp][par][:], Exp, bias=0.0, scale=0.125
                        )
                        ets[hp][par] = et
                # --- phase B: av (cols 0:256) + denominator (cols 256:512)
                # in one bank, col-tiled 128x64 T0/T1 ---
                for hp in range(H // 2):
                    ad = ad_pool.tile([128, 512], F32, tag="ad", name="ad")
                    for par in range(2):
                        hr = par * 64
                        h = 2 * hp + par
                        for kc in range(2):
                            tkc = blk * 2 + kc
                            nc.tensor.matmul(
                                ad[hr:hr + 64, 0:256],
                                v_sb[:, tkc, h * 64:(h + 1) * 64],
                                ets[hp][par][:, kc],
                                start=(kc == 0), stop=(kc == 1),
                            )
                    for par in range(2):
                        hr = par * 64
                        for kc in range(2):
                            tkc = blk * 2 + kc
                            nc.tensor.matmul(
                                ad[hr:hr + 64, 256:512],
                                mbc_sb[:, tkc],
                                ets[hp][par][:, kc],
                                start=(kc == 0), stop=(kc == 1),
                            )
                    rc = nrm_pool.tile([128, 256], F32, tag="rc", name="rc")
                    nc.vector.reciprocal_approx_fast(rc[:], ad[:, 256:512])
                    nc.vector.tensor_mul(
                        at_sb[:, hp, q0:q0 + 256], ad[:, 0:256], rc[:]
                    )

            def ff(tt):
                for oc in range(DC):
                    ps = mm_pool.tile([128, 512], F32, tag="mm", name="ps")
                    for hc in range(HC):
                        nc.tensor.matmul(
                            ps[:],
                            wff_sb[:, hc, oc * 128:(oc + 1) * 128],
                            at_sb[:, hc, tt * 512:(tt + 1) * 512],
                            start=(hc == 0),
                            stop=(hc == HC - 1),
                        )
                    ob = ob_pool.tile([128, 512], F32, tag="ob", name="ob")
                    nc.scalar.activation(
                        ob[:], ps[:], Ident, bias=bffe_sb[:, oc:oc + 1], scale=1.0
                    )
                    nc.sync.dma_start(
                        out=out_d.ap()[oc * 128:(oc + 1) * 128,
                                       tt * 512:(tt + 1) * 512],
                        in_=ob[:],
                    )

            # ---- software-pipelined emission over 4 token-quarters ----
            for u in range(4):
                proj_qk(wq_sb, bq_sb, qt_sb, u)
                proj_qk(wk_sb, bk_sb, kt_sb, u)
                for kc in range(4 * u, 4 * u + 4):
                    proj_v(kc)
                attn_block(2 * u)
                attn_block(2 * u + 1)
                ff(u)

    nc.finalize()
    return nc


def _get_nc():
    if "nc" not in _nc_cache:
        _nc_cache["nc"] = _build_nc()
    return _nc_cache["nc"]


def _prep_in_maps(X, mask, Wq, bq, Wk, bk, Wv, bv, Wff, bff):
    bf = ml_dtypes.bfloat16
    wq_b = np.ascontiguousarray(Wq.astype(bf))
    wk_b = np.ascontiguousarray(Wk.astype(bf))
    wv_b = np.ascontiguousarray(Wv.astype(bf))
    wff_b = np.ascontiguousarray(Wff.astype(bf))
    # per-partition bias layouts: [128, nchunks] with col = chunk
    bq_t = np.ascontiguousarray(bq.astype(np.float32).reshape(HC, 128).T)
    bk_t = np.ascontiguousarray(bk.astype(np.float32).reshape(HC, 128).T)
    bffe = (bff.astype(np.float64)
            + bv.astype(np.float64) @ Wff.astype(np.float64)).astype(np.float32)
    bffe_t = np.ascontiguousarray(bffe.reshape(DC, 128).T)

    in_maps = []
    for c in range(NCORES):
        b, s0 = divmod(c, 2)
        s0 *= T
        xt = np.ascontiguousarray(X[b, s0:s0 + T, :].T.astype(bf))
        mvalid = (mask[b, s0:s0 + T] > 0).astype(np.float32)  # [T] 0/1
        mv_t = np.ascontiguousarray(mvalid.reshape(NKC, 128).T)  # [128, NKC]
        mbc = np.ascontiguousarray(
            np.broadcast_to(mv_t[:, :, None], (128, NKC, 64))
            .reshape(128, NKC * 64).astype(bf))
        in_maps.append({
            "xt": xt, "wq": wq_b, "wk": wk_b, "wv": wv_b, "wff": wff_b,
            "bq": bq_t, "bk": bk_t, "bffe": bffe_t,
            "mv": mv_t, "mbc": mbc,
        })
    return in_maps


def _assemble(results):
    out = np.empty((B, S, DIM), np.float32)
    for c in range(NCORES):
        b, s0 = divmod(c, 2)
        s0 *= T
        out[b, s0:s0 + T, :] = results[c]["out"].T
    return out


def run(trace=False, **inputs):
    nc = _get_nc()
    in_maps = _prep_in_maps(**inputs)
    res = run_bass_kernel_spmd(
        nc, in_maps, core_ids=list(range(NCORES)), trace=trace
    )
    return _assemble(res.results), res


def kernel(**inputs) -> np.ndarray:
    out, _ = run(trace=False, **inputs)
    return out
